# revision 32
# baseline (speedup 1.0000x reference)
"""Trainium2 Bass kernel for nn_LOCATE (spatial+temporal attention).

Data-parallel over batch: B=64 -> 8 per core on 8 NeuronCores.
Per core (b_local=8):
  v = obj @ s_wv_w.T ; score = tanh(v + h) @ s_wa ; alpha = softmax_n(score)
  obj_att = alpha @ obj ; feat = [obj_att, frame]
  v2 = feat @ t_wv_w.T ; score2 = tanh(v2 + h2) @ t_wa ; beta = softmax_f(score2)
  out = beta @ feat

Key layout decisions:
- All transposes happen on the host: obj arrives both as objT (contraction
  dim on partitions, fp8, scaled 1/8) for the big PE matmul and as objN
  (rows on partitions, bf16) for the alpha-weighted reduction.
- The dominant GEMM (72k x 1024 x 1024 per core) runs in fp8 e4m3 with
  DoubleRow perf mode (2 k-tiles contracted per instruction, halving
  instruction count and PE time vs bf16). Each (a-tile, k-pair) streams
  a whole batch into one 3-bank PSUM tile as 512/512/128-col chunks
  (matmul outs must stay within a 2KB PSUM bank).
- tanh is written as fp8 (x1 scale) and the score matmul runs fp8
  DoubleRow against s_wa x8; exp() descales by 1/8 via the ACT scale.
- obj_att runs on the PE: alpha is transposed once per batch and placed
  into masked [128 x 32] stationary matrices (box->frame membership
  masks precomputed on host), so the weighted box-sum is 18 matmuls.
- The batch loop is software-pipelined: batch b+1's GEMM phase is
  emitted before batch b's softmax/obj_att phase so the PE never waits
  on the DVE/ACT softmax chain.
"""

import numpy as np
import ml_dtypes
from contextlib import ExitStack

import concourse.bass as bass
import concourse.bacc as bacc
import concourse.tile as tile
from concourse import mybir
from concourse.bass_utils import run_bass_kernel_spmd

F32 = mybir.dt.float32
BF16 = mybir.dt.bfloat16
F8 = mybir.dt.float8e4
TANH = mybir.ActivationFunctionType.Tanh
EXP = mybir.ActivationFunctionType.Exp
MULT = mybir.AluOpType.mult
ADD = mybir.AluOpType.add
DR = mybir.MatmulPerfMode.DoubleRow
AXN = mybir.AxisListType.X

B_LOC = 8          # batches per core
F = 32             # frames
N = 36             # boxes
K = 1024           # REGION = HIDDEN = ATT
K2 = 3072          # FEAT2
MB = F * N         # 1152 rows per batch
MT = 384           # m-chunk for tanh/score (3 per batch, batch-aligned)
BF = B_LOC * F     # 256
NCORES = 8
S = 8.0            # fp8 pre-scale on weights (1/S on obj)

_CACHE = {}


def _build():
    nc = bacc.Bacc("TRN2", target_bir_lowering=False, debug=False,
                   num_devices=NCORES)

    objT8 = nc.declare_dram_parameter("objT8", [128, B_LOC, 8, MB], F8, isOutput=False)
    objN = nc.declare_dram_parameter("objN", [128, B_LOC, 9, K], BF16, isOutput=False)
    frameT = nc.declare_dram_parameter("frameT", [128, 16, BF], BF16, isOutput=False)
    hidT = nc.declare_dram_parameter("hidT", [128, 8, B_LOC], BF16, isOutput=False)
    swvT8 = nc.declare_dram_parameter("swvT8", [128, 8, K], F8, isOutput=False)
    swhT = nc.declare_dram_parameter("swhT", [128, 8, K], BF16, isOutput=False)
    twhT = nc.declare_dram_parameter("twhT", [128, 8, K], BF16, isOutput=False)
    twvT = nc.declare_dram_parameter("twvT", [128, 24, K], BF16, isOutput=False)
    # wa pairs replicated 16x: [p, atp, ktile(2), rep(16)] — narrow fp8
    # DR ldweights (2-col) are invalid ISA; 32-byte rows compile
    wa8 = nc.declare_dram_parameter("wa8", [128, 4, 2, 16], F8, isOutput=False)
    twa = nc.declare_dram_parameter("twa", [128, 8], BF16, isOutput=False)
    combo1 = nc.declare_dram_parameter("combo1", [128, 8], F32, isOutput=False)
    combo2 = nc.declare_dram_parameter("combo2", [128, 8], F32, isOutput=False)
    id16 = nc.declare_dram_parameter("id16", [128, 128], BF16, isOutput=False)
    id32 = nc.declare_dram_parameter("id32", [128, 128], F32, isOutput=False)
    ones16 = nc.declare_dram_parameter("ones16", [1, 128], BF16, isOutput=False)
    masks = nc.declare_dram_parameter("masks", [128, 9, F], BF16, isOutput=False)
    # transposed output [k-part, ktile, b]; host untransposes (cheap)
    out = nc.declare_dram_parameter("out", [128, 24, B_LOC], BF16, isOutput=True)

    with ExitStack() as ctx, nc.allow_low_precision("bf16/fp8 attention"):
        tc = ctx.enter_context(tile.TileContext(nc))

        wpool = ctx.enter_context(tc.tile_pool(name="weights", bufs=1))
        objp = ctx.enter_context(tc.tile_pool(name="objp", bufs=2))
        objnp = ctx.enter_context(tc.tile_pool(name="objnp", bufs=2))
        thp = ctx.enter_context(tc.tile_pool(name="thp", bufs=2))
        small = ctx.enter_context(tc.tile_pool(name="small", bufs=2))
        pvb = ctx.enter_context(tc.tile_pool(name="pvb", bufs=2, space="PSUM"))
        aux = ctx.enter_context(tc.tile_pool(name="aux", bufs=2, space="PSUM"))

        def load(pool, dram, shape, dt, tag=""):
            t = pool.tile(shape, dt, tag=tag, name=f"ld_{dram.name}")
            nc.sync.dma_start(out=t[:], in_=dram[:])
            return t

        # ---- front weights / consts. DMA issue order is the priority
        # order: swvT8 + objT8[0] unblock the batch-0 GEMM immediately,
        # swhT feeds the h-projection (hoisted into batch 0's GEMM loop).
        # twhT/frameT/twvT are only needed late -> issued in the loop. ----
        hidT_sb = load(wpool, hidT, [128, 8, B_LOC], BF16)
        swvT8_sb = load(wpool, swvT8, [128, 8, K], F8)
        o8_first = objp.tile([128, 8, MB], F8, tag="obj8", name="o8")
        nc.sync.dma_start(out=o8_first[:], in_=objT8[:, 0])
        id32_sb = load(wpool, id32, [128, 128], F32)
        combo1_sb = load(wpool, combo1, [128, 8], F32)
        # column-split so the first h-projection half starts ~3.5us sooner
        swhT_sb = objnp.tile([128, 8, K], BF16, tag="objn", name="ld_swhT")
        nc.sync.dma_start(out=swhT_sb[:, :, 0:512], in_=swhT[:, :, 0:512])
        nc.sync.dma_start(out=swhT_sb[:, :, 512:K], in_=swhT[:, :, 512:K])
        wa8_sb = load(wpool, wa8, [128, 4, 2, 16], F8)
        combo2_sb = load(wpool, combo2, [128, 8], F32)
        id16_sb = load(wpool, id16, [128, 128], BF16)
        ones16_sb = load(wpool, ones16, [1, 128], BF16)
        masks_sb = load(wpool, masks, [128, 9, F], BF16)
        twa_sb = load(wpool, twa, [128, 8], BF16)

        featT = wpool.tile([128, 24, BF], BF16)  # [k-part, ktile, b*F+f]
        hTa = wpool.tile([128, 8, B_LOC], F32)
        h2Ta = wpool.tile([128, 8, B_LOC], F32)

        def h_projection(dst, wsb, cmb):
            # per column-half: project, then immediately transpose+bias its
            # four a-tiles, so dst[at=0] (which gates batch 0's first tanh
            # and the GEMM PSUM ring) is ready as early as possible
            ph_sb = small.tile([B_LOC, K], F32, tag="ph", bufs=1,
                               name="ph_sb")
            for hh in range(2):
                php = aux.tile([B_LOC, 512], F32, tag="aux", name="php")
                for kt in range(8):
                    nc.tensor.matmul(php[:], hidT_sb[:, kt, :],
                                     wsb[:, kt, hh * 512:(hh + 1) * 512],
                                     start=(kt == 0), stop=(kt == 7))
                nc.vector.tensor_copy(ph_sb[:, hh * 512:(hh + 1) * 512],
                                      php[:])
                for at in range(4 * hh, 4 * hh + 4):
                    pt = aux.tile([128, B_LOC], F32, tag="aux", name="pt")
                    nc.tensor.transpose(pt[:],
                                        ph_sb[:, at * 128:(at + 1) * 128],
                                        id32_sb[0:B_LOC, 0:B_LOC])
                    nc.vector.tensor_scalar_add(dst[:, at, :], pt[:],
                                                cmb[:, at:at + 1])

        # ================= software-pipelined batch loop =================
        state = {}

        def phase_A(b):
            """DMA + main fp8 GEMM + tanh for batch b."""
            if b == 0:
                o8 = o8_first
            else:
                o8 = objp.tile([128, 8, MB], F8, tag="obj8", name="o8")
                nc.sync.dma_start(out=o8[:], in_=objT8[:, b])
            on = objnp.tile([128, 9, K], BF16, tag="objn", name="on")
            nc.sync.dma_start(out=on[:], in_=objN[:, b])
            # big weight loads are spread evenly across batches 1-6 in
            # ~1-1.5MB chunks so they never lag the per-batch obj DMAs
            if b == 1:
                state["twhT_sb"] = wpool.tile([128, 8, K], BF16,
                                              name="twhT_sb")
            if 1 <= b <= 2:
                c = 4 * (b - 1)
                nc.sync.dma_start(out=state["twhT_sb"][:, c:c + 4, :],
                                  in_=twhT[:, c:c + 4, :])
            if b == 3:
                nc.sync.dma_start(out=featT[:, 8:24, :], in_=frameT[:])
                state["twvT_sb"] = wpool.tile([128, 24, K], BF16,
                                              name="twvT_sb")
            if 3 <= b <= 6:
                c = 6 * (b - 3)
                nc.sync.dma_start(out=state["twvT_sb"][:, c:c + 6, :],
                                  in_=twvT[:, c:c + 6, :])

            th = thp.tile([128, 8, MB], F8, tag="tanh", name="th")
            for a in range(8):
                # one 3-bank PSUM tile per a-tile; each matmul out stays
                # inside a bank (<=512 fp32), same weights for all 3 chunks
                pj = pvb.tile([128, 3 * 512], F32, tag="pvb", name="pj")
                for tp in range(4):
                    for c0, c1 in ((0, 512), (512, 1024), (1024, MB)):
                        nc.tensor.matmul(
                            pj[:, c0:c1],
                            swvT8_sb[:, 2 * tp:2 * tp + 2,
                                     a * 128:(a + 1) * 128],
                            o8[:, 2 * tp:2 * tp + 2, c0:c1],
                            start=(tp == 0), stop=(tp == 3),
                            perf_mode=DR, skip_group_check=True)
                nc.scalar.activation(th[:, a, :], pj[:, 0:MB], TANH,
                                     bias=hTa[:, a, b:b + 1], scale=1.0)
                if b == 0 and a == 1:
                    # hoisted here so batch 0's GEMM hides the projection;
                    # must precede a==2 (pj ring wrap waits on tanh(a=0))
                    h_projection(hTa, swhT_sb, combo1_sb)
            state[b] = (on, th)

        def phase_B(b):
            """score + softmax + obj_att + featT fill for batch b."""
            on, th = state.pop(b)

            # score = wa.T @ th (fp8 DoubleRow); exp straight out of PSUM
            erow = small.tile([1, MB], BF16, tag="erow", name="erow")
            for j in range(3):
                sp = aux.tile([16, MT], F32, tag="aux", name="sp")
                for atp in range(4):
                    nc.tensor.matmul(sp[:], wa8_sb[:, atp],
                                     th[:, 2 * atp:2 * atp + 2,
                                        j * MT:(j + 1) * MT],
                                     start=(atp == 0), stop=(atp == 3),
                                     perf_mode=DR, skip_group_check=True)
                nc.scalar.activation(erow[:, j * MT:(j + 1) * MT],
                                     sp[0:1, :], EXP, scale=1.0 / S)
            # alpha onto partitions first: these transposes depend only on
            # erow (ready early), so the PE runs them while the DVE computes
            # the softmax denominators for the prB broadcast below
            sums = small.tile([1, F], F32, tag="sums", name="sums")
            nc.vector.reduce_sum(sums[:],
                                 erow[:].rearrange("p (f n) -> p f n", n=N),
                                 axis=AXN)
            rec = small.tile([1, F], BF16, tag="rec", name="rec")
            nc.vector.reciprocal(rec[:], sums[:])
            pat = aux.tile([128, 32], BF16, tag="aux", name="pat")
            patv = pat[:].rearrange("p (m two) -> p m two", two=2)
            for mt in range(9):
                nc.tensor.transpose(patv[:, mt, 0:1],
                                    erow[:, mt * 128:(mt + 1) * 128],
                                    id16_sb[0:1, 0:1])
            eT = small.tile([128, 16], BF16, tag="eT", name="eT")
            nc.vector.tensor_copy(eT[:, 0:9], patv[:, 0:9, 0])
            prB = aux.tile([128, F], F32, tag="aux", name="prB")
            nc.tensor.matmul(prB[:], ones16_sb[:], rec[:], start=True,
                             stop=True)
            recB = small.tile([128, F], BF16, tag="recB", name="recB")
            nc.vector.tensor_copy(recB[:], prB[:])

            # masked, normalized alpha as stationary matrices [128, 9, F]
            mrec = small.tile([128, 9, F], BF16, tag="mrec", name="mrec")
            m0, m1 = bass.broadcast_tensor_aps(masks_sb[:], recB[:, None, :])
            nc.vector.tensor_tensor(mrec[:], m0, m1, op=MULT)
            alphaM = small.tile([128, 9, F], BF16, tag="alphaM", name="alphaM")
            a0, a1 = bass.broadcast_tensor_aps(mrec[:], eT[:, 0:9, None])
            nc.vector.tensor_tensor(alphaM[:], a0, a1, op=MULT)

            # obj_att[f, d] on the PE (both d-halves under each stationary
            # alphaM load), then transpose into featT
            oa_sb = small.tile([F, K], BF16, tag="oa", name="oa_sb")
            pao = [aux.tile([F, 512], F32, tag="aux", name=f"pao{hh}")
                   for hh in range(2)]
            for mt in range(9):
                for hh in range(2):
                    nc.tensor.matmul(pao[hh][:], alphaM[:, mt, :],
                                     on[:, mt, hh * 512:(hh + 1) * 512],
                                     start=(mt == 0), stop=(mt == 8),
                                     skip_group_check=True)
            for hh in range(2):
                nc.vector.tensor_copy(oa_sb[:, hh * 512:(hh + 1) * 512],
                                      pao[hh][:])
            pft = aux.tile([128, 8, F], BF16, tag="aux", name="pft")
            for kt in range(8):
                nc.tensor.transpose(pft[:, kt, :],
                                    oa_sb[0:F, kt * 128:(kt + 1) * 128],
                                    id16_sb[0:F, 0:F])
            nc.vector.tensor_copy(featT[:, 0:8, b * F:(b + 1) * F], pft[:])
            if b == 4:
                h_projection(h2Ta, state["twhT_sb"], combo2_sb)

        for b in range(B_LOC + 1):
            if b < B_LOC:
                phase_A(b)
            if b >= 1:
                phase_B(b - 1)

        # ================= temporal attention =================
        twvT_sb = state["twvT_sb"]
        tanh2 = wpool.tile([128, 8, BF], BF16)
        for a in range(8):
            pv2 = aux.tile([128, 512], F32, tag="aux", name="pv2")
            for kt in range(24):
                nc.tensor.matmul(pv2[:, 0:BF],
                                 twvT_sb[:, kt, a * 128:(a + 1) * 128],
                                 featT[:, kt, :],
                                 start=(kt == 0), stop=(kt == 23))
            tin = small.tile([128, BF], BF16, tag="tin", name="tin")
            c0, c1 = bass.broadcast_tensor_aps(
                pv2[:, 0:BF].rearrange("p (b f) -> p b f", f=F),
                h2Ta[:, a, :, None])
            nc.vector.tensor_tensor(tin[:].rearrange("p (b f) -> p b f", f=F),
                                    c0, c1, op=ADD)
            nc.scalar.activation(tanh2[:, a, :], tin[:], TANH)

        sp2 = aux.tile([1, BF], F32, tag="aux", name="sp2")
        for at in range(8):
            nc.tensor.matmul(sp2[:], twa_sb[:, at:at + 1], tanh2[:, at, :],
                             start=(at == 0), stop=(at == 7))
        e2 = small.tile([1, BF], BF16, tag="e2", name="e2")
        nc.scalar.activation(e2[:], sp2[:], EXP)
        sums2 = small.tile([1, B_LOC], F32, tag="sums2", name="sums2")
        nc.vector.reduce_sum(sums2[:],
                             e2[:].rearrange("p (b f) -> p b f", f=F), axis=AXN)
        rec2 = small.tile([1, B_LOC], BF16, tag="rec2", name="rec2")
        nc.vector.reciprocal(rec2[:], sums2[:])
        b2 = small.tile([1, BF], BF16, tag="b2", name="b2")
        d0, d1 = bass.broadcast_tensor_aps(
            e2[:].rearrange("p (b f) -> p b f", f=F), rec2[:, :, None])
        nc.vector.tensor_tensor(b2[:].rearrange("p (b f) -> p b f", f=F),
                                d0, d1, op=MULT)
        pbB = aux.tile([128, BF], F32, tag="aux", name="pbB")
        nc.tensor.matmul(pbB[:], ones16_sb[:], b2[:], start=True, stop=True)
        bB = small.tile([128, BF], BF16, tag="bB", name="bB")
        nc.vector.tensor_copy(bB[:], pbB[:])

        # --- loc = sum_f beta * feat, kept transposed; host untransposes ---
        locT = wpool.tile([128, 24, B_LOC], BF16)
        for kt in range(0, 24, 4):
            tmp = small.tile([128, 4, BF], BF16, tag="tmp", name="tmp")
            f0, f1 = bass.broadcast_tensor_aps(featT[:, kt:kt + 4, :],
                                               bB[:, None, :])
            nc.vector.tensor_tensor(tmp[:], f0, f1, op=MULT)
            nc.vector.reduce_sum(
                locT[:, kt:kt + 4, :],
                tmp[:].rearrange("p k (b f) -> p (k b) f", f=F),
                axis=AXN)
        nc.sync.dma_start(out=out[:], in_=locT[:])

    nc.compile()
    return nc


def _prep(inputs):
    bf = ml_dtypes.bfloat16
    f8 = ml_dtypes.float8_e4m3
    f32 = np.float32

    def rT(w, nt):  # [a,k] torch-linear -> [128, nt, a] partition-major W.T
        return np.ascontiguousarray(
            w.T.reshape(nt, 128, -1).transpose(1, 0, 2))

    mvec = np.arange(MB) // N  # frame index of each (f,n) row
    shared = {
        "swvT8": rT(np.asarray(inputs["s_wv_w"], f32) * S, 8).astype(f8),
        "swhT": rT(np.asarray(inputs["s_wh_w"], f32), 8).astype(bf),
        "twhT": rT(np.asarray(inputs["t_wh_w"], f32), 8).astype(bf),
        "twvT": rT(np.asarray(inputs["t_wv_w"], f32), 24).astype(bf),
        # [p, atp, ktile, rep4]: wa8[p, atp, r, i] = s_wa[(2*atp+r)*128+p]*S
        "wa8": np.ascontiguousarray(
            np.broadcast_to(
                (np.asarray(inputs["s_wa_w"], f32) * S)
                .reshape(4, 2, 128).transpose(2, 0, 1)[:, :, :, None],
                (128, 4, 2, 16))).astype(f8),
        "twa": np.ascontiguousarray(
            np.asarray(inputs["t_wa_w"], f32).reshape(8, 128).T).astype(bf),
        "combo1": np.ascontiguousarray(
            (np.asarray(inputs["s_wv_b"], f32)
             + np.asarray(inputs["s_wh_b"], f32)).reshape(8, 128).T),
        "combo2": np.ascontiguousarray(
            (np.asarray(inputs["t_wv_b"], f32)
             + np.asarray(inputs["t_wh_b"], f32)).reshape(8, 128).T),
        "id16": np.eye(128).astype(bf),
        "id32": np.eye(128, dtype=f32),
        "ones16": np.ones((1, 128)).astype(bf),
        "masks": np.ascontiguousarray(
            (mvec.reshape(9, 128).T[:, :, None]
             == np.arange(F)[None, None, :])).astype(bf),
    }
    objf = np.asarray(inputs["object_feats"], f32)
    frm = np.asarray(inputs["frame_feats"], f32)
    hid = np.asarray(inputs["hidden_state"], f32)
    in_maps = []
    for c in range(NCORES):
        sl = slice(c * B_LOC, (c + 1) * B_LOC)
        obm = objf[sl].reshape(B_LOC, MB, K)
        m = dict(shared)
        m["objT8"] = np.ascontiguousarray(
            (obm * (1.0 / S)).astype(f8).reshape(B_LOC, MB, 8, 128)
            .transpose(3, 0, 2, 1))
        m["objN"] = np.ascontiguousarray(
            obm.astype(bf).reshape(B_LOC, 9, 128, K).transpose(2, 0, 1, 3))
        m["frameT"] = np.ascontiguousarray(
            frm[sl].astype(bf).reshape(B_LOC, F, 16, 128)
            .transpose(3, 2, 0, 1).reshape(128, 16, BF))
        m["hidT"] = np.ascontiguousarray(
            hid[sl].astype(bf).reshape(B_LOC, 8, 128).transpose(2, 1, 0))
        in_maps.append(m)
    return in_maps


def kernel(**inputs):
    if "nc" not in _CACHE:
        _CACHE["nc"] = _build()
    in_maps = _prep(inputs)
    res = run_bass_kernel_spmd(_CACHE["nc"], in_maps,
                               core_ids=list(range(NCORES)))
    _CACHE["last_exec_ns"] = res.exec_time_ns
    if res.instructions_and_trace:
        _CACHE["last_trace"] = res.instructions_and_trace[1]
    # out arrives as locT [128, 24, B_LOC]; untranspose to [B_LOC, 3072]
    return np.concatenate(
        [np.asarray(res.results[c]["out"]).astype(np.float32)
         .transpose(2, 1, 0).reshape(B_LOC, K2)
         for c in range(NCORES)], axis=0)
